# revision 39
# baseline (speedup 1.0000x reference)
"""Multi-head attention (B=4, N=2048, D=768, H=12) on 8 trn2 NeuronCores.

Sharding: core c -> (batch b = c//2, head-half g = c%2).  Each core computes
the qkv projection for its 6 heads, attention, and a partial output
projection (over its 384 feature columns).  The host sums the two partials
per batch and adds the proj bias.  No collectives.

v3 design notes (from HW microbenchmarks):
 - Matmuls stream 1 cyc/col at 2.4 GHz only while the PE is continuously
   fed; idle gaps drop the PE to a lower p-state (~2x) and >3.4us gaps
   re-trigger HAM throttling.  The attention inner loop is ACT-bound
   (exp [128,1024] = 1114 ns vs ~639 ns of PE work per k-tile), so all
   non-attention matmuls (QKV, V, proj) are interleaved as fine-grained
   fillers inside the attention loops to keep the PE warm and busy.
 - Loop grid is pair-major: for pr in 0..2: for qc in 0..3.  Fillers are
   placed with explicit deadlines (Q/K of pair p+1 finish during pair p's
   loops; proj(qc) runs during (pr2, qc+1)).
 - All matmul operands bf16 (PSUM stays fp32): halves SBUF traffic + DMA.
 - V stored flat per (kt, pair) as [Vh0(64)|1|Vh1(64)|pad|1] (131 wide);
   AV uses M=128 lhsT windows (h0: +0, h1: +65) so FWL stays on, softmax
   denominators appear at PSUM row 64 (h0) / row 65 (h1), and junk rows
   are never read.  Denominators are repacked to partitions 64/65 of one
   bank so ONE [2,512] DVE reciprocal serves both heads.
 - proj results DMA directly PSUM -> DRAM (no SBUF staging).
 - PSUM: 2x S^T double-buffer (4 banks) + AV accum (2) + aux pool (2).
"""

import numpy as np
import ml_dtypes

import concourse.bacc as bacc
import concourse.bass as bass  # noqa: F401
import concourse.mybir as mybir
import concourse.tile as tile
from concourse.bass_utils import run_bass_kernel_spmd

P = 128
NQ = 2048          # sequence length
CD = 768           # model dim
NHC = 6            # heads per core
DH = 64            # head dim
SCALE = DH ** -0.5
CT = CD // P       # 6 c-tiles
KT = NQ // P       # 16 k-tiles
QC = 512           # q chunk
NQC = NQ // QC     # 4
PAIRS = NHC // 2   # 3
VB = 2 * DH + 3    # 131: flat v block width per (kt, pair)
VW = KT * PAIRS * VB + DH + 2  # + tail pad so the last h1 window is in-bounds

F32 = mybir.dt.float32
BF16 = mybir.dt.bfloat16


def build_nc(n_reps=1, debug=False):
    nc = bacc.Bacc("TRN2", debug=False, num_devices=8)

    xT_d = nc.dram_tensor("xT", [CD, NQ], BF16, kind="ExternalInput")
    wqkvT_d = nc.dram_tensor("wqkvT", [CD, 3 * 384], BF16, kind="ExternalInput")
    bqk_d = nc.dram_tensor("b_qk", [P, 6], F32, kind="ExternalInput")
    bv_d = nc.dram_tensor("b_v", [1, 384], BF16, kind="ExternalInput")
    wpT_d = nc.dram_tensor("wpT", [384, CD], BF16, kind="ExternalInput")
    ones_d = nc.dram_tensor("ones", [P, P], BF16, kind="ExternalInput")
    out_d = nc.dram_tensor("out", [NQ, CD], F32, kind="ExternalOutput")
    if debug:
        qk_dbg = nc.dram_tensor("qk_dbg", [2, PAIRS, P, NQ], BF16,
                                kind="ExternalOutput")
        v_dbg = nc.dram_tensor("v_dbg", [P, VW], BF16, kind="ExternalOutput")
        at_dbg = nc.dram_tensor("at_dbg", [NQC, P, PAIRS, QC], BF16,
                                kind="ExternalOutput")
        den_dbg = nc.dram_tensor("den_dbg", [PAIRS, NQC, 33, QC], BF16,
                                 kind="ExternalOutput")

    with tile.TileContext(nc) as tc:
        with (
            tc.tile_pool(name="consts", bufs=1) as consts,
            tc.tile_pool(name="big", bufs=1) as big,
            tc.tile_pool(name="attn", bufs=1) as attn_pool,
            tc.tile_pool(name="aT", bufs=4) as aT_pool,
            tc.tile_pool(name="norm", bufs=1) as norm_pool,
            tc.tile_pool(name="outst", bufs=2) as outst_pool,
            tc.tile_pool(name="ps_s", bufs=2, space="PSUM") as ps_s,
            tc.tile_pool(name="ps_av", bufs=1, space="PSUM") as ps_av,
            tc.tile_pool(name="aux", bufs=1, space="PSUM") as aux,
        ):
            # ---- constants ----
            # single tiles with a c-tile dim; DMA issue costs ~650ns each on
            # ANY queue, so batch the input loads into a few big DMAs (one
            # per xT chunk / weight piece) using partition-regrouped dram
            # views, alternating the sync/gpsimd rings
            xT_all = consts.tile([P, CT, NQ], BF16, tag="xT", name="xT_all")
            wq_all = consts.tile([P, CT, 3 * 384], BF16, tag="wq", name="wq_all")
            xT_sb = [xT_all[:, ct, :] for ct in range(CT)]
            wq_sb = [wq_all[:, ct, :] for ct in range(CT)]
            xTv = xT_d.rearrange("(a p) n -> p a n", p=P)
            wqv = wqkvT_d.rearrange("(a p) n -> p a n", p=P)
            nc.gpsimd.dma_start(wq_all[:, :, 384:768], wqv[:, :, 384:768])
            nc.sync.dma_start(xT_all[:, :, 0:QC], xTv[:, :, 0:QC])
            nc.gpsimd.dma_start(wq_all[:, :, 0:384], wqv[:, :, 0:384])
            nc.sync.dma_start(wq_all[:, :, 768:1152], wqv[:, :, 768:1152])
            nc.gpsimd.dma_start(xT_all[:, :, QC:2 * QC], xTv[:, :, QC:2 * QC])
            nc.sync.dma_start(xT_all[:, :, 2 * QC:3 * QC],
                              xTv[:, :, 2 * QC:3 * QC])
            nc.gpsimd.dma_start(xT_all[:, :, 3 * QC:4 * QC],
                                xTv[:, :, 3 * QC:4 * QC])
            wp_all = consts.tile([P, 3, CD], BF16, tag="wpT", name="wp_all")
            wp_sb = [wp_all[:, t3, :] for t3 in range(3)]
            nc.sync.dma_start(
                wp_all[:, :, :], wpT_d.rearrange("(a p) n -> p a n", p=P))
            bqk_sb = consts.tile([P, 6], F32, tag="bqk")
            nc.gpsimd.dma_start(bqk_sb[:, :], bqk_d[:, :])
            bv_sb = consts.tile([1, 384], BF16, tag="bv")
            nc.gpsimd.dma_start(bv_sb[:, :], bv_d[:, :])
            ones_sb = consts.tile([P, P], BF16, tag="ones")
            nc.gpsimd.dma_start(ones_sb[:, :], ones_d[:, :])
            # -ln(64) exp bias (softmax-invariant; keeps 1/sum well scaled)
            expb_sb = consts.tile([P, 1], F32, tag="expb")
            nc.vector.memset(expb_sb[:, :], -4.1588830833596715)

            for _rep in range(n_reps):
                # ---- persistent activations ----
                # per-pair Q^T/K^T [128, 2048]: rows 0-63 head 2p, 64-127 head 2p+1
                q_sb = [big.tile([P, NQ], BF16, tag=f"q{p}", name=f"q{p}")
                        for p in range(PAIRS)]
                k_sb = [big.tile([P, NQ], BF16, tag=f"k{p}", name=f"k{p}")
                        for p in range(PAIRS)]
                # flat v: per (kt, pair) block [Vh0|1|Vh1|pad|1], width 131
                v_sb = big.tile([P, VW], BF16, tag="v")
                nc.vector.memset(v_sb[:, :], 0.0)
                vv = v_sb[:, 0:KT * PAIRS * VB].rearrange(
                    "p (a b w) -> p a b w", a=KT, b=PAIRS)
                # ones columns for the softmax denominators
                nc.gpsimd.dma_start(
                    vv[:, :, :, DH],
                    ones_d[:, 0:KT * PAIRS].rearrange("p (a b) -> p a b", a=KT))
                nc.gpsimd.dma_start(
                    vv[:, :, :, 2 * DH + 2],
                    ones_d[:, 0:KT * PAIRS].rearrange("p (a b) -> p a b", a=KT))

                # ---------------- building blocks ----------------

                def qk_pieces(kind, t, qc):
                    # Q^T (kind 0) / K^T (kind 1) pair-tile t, one 512-chunk,
                    # split into two 3-matmul pieces sharing one aux psum.
                    dest = (q_sb if kind == 0 else k_sb)[t]
                    col0 = kind * 384 + t * P
                    st = {}

                    def p1():
                        st["ps"] = aux.tile([P, QC], F32, tag="aux", name="qkvps")
                        for ct in range(3):
                            nc.tensor.matmul(
                                st["ps"][:, :],
                                lhsT=wq_sb[ct][:, col0:col0 + P],
                                rhs=xT_sb[ct][:, qc * QC:(qc + 1) * QC],
                                start=(ct == 0), stop=False)

                    def p2():
                        for ct in range(3, CT):
                            nc.tensor.matmul(
                                st["ps"][:, :],
                                lhsT=wq_sb[ct][:, col0:col0 + P],
                                rhs=xT_sb[ct][:, qc * QC:(qc + 1) * QC],
                                start=False, stop=(ct == CT - 1))
                        with nc.allow_low_precision(reason="bf16 q/k"):
                            nc.vector.tensor_scalar_add(
                                out=dest[:, qc * QC:(qc + 1) * QC],
                                in0=st["ps"][:, :],
                                scalar1=bqk_sb[:, kind * 3 + t:kind * 3 + t + 1])

                    return [p1, p2]

                def v_pieces(nt):
                    st = {}

                    def p1():
                        st["ps"] = aux.tile([P, QC], F32, tag="aux", name="qkvps")
                        for ct in range(3):
                            nc.tensor.matmul(
                                st["ps"][:, 0:384],
                                lhsT=xT_sb[ct][:, nt * P:(nt + 1) * P],
                                rhs=wq_sb[ct][:, 768:1152],
                                start=(ct == 0), stop=False)

                    def p2():
                        for ct in range(3, CT):
                            nc.tensor.matmul(
                                st["ps"][:, 0:384],
                                lhsT=xT_sb[ct][:, nt * P:(nt + 1) * P],
                                rhs=wq_sb[ct][:, 768:1152],
                                start=False, stop=False)
                        nc.tensor.matmul(
                            st["ps"][:, 0:384],
                            lhsT=ones_sb[0:1, :],
                            rhs=bv_sb[0:1, :],
                            start=False, stop=True)
                        src = st["ps"][:, 0:384].rearrange(
                            "p (b c d) -> p b c d", b=PAIRS, c=2)
                        with nc.allow_low_precision(reason="bf16 v"):
                            nc.vector.tensor_copy(
                                out=vv[:, nt, :, 0:DH],
                                in_=src[:, :, 0, :])
                            nc.vector.tensor_copy(
                                out=vv[:, nt, :, DH + 1:2 * DH + 1],
                                in_=src[:, :, 1, :])

                    return [p1, p2]

                at_chunks = {}

                def proj_pieces(qc):
                    at_chunk = at_chunks[qc]
                    pieces = []
                    for sub in range(QC // P):
                        for (o0, ow) in ((0, 512), (512, 256)):
                            def piece(sub=sub, o0=o0, ow=ow):
                                pp = aux.tile([P, QC], F32, tag="aux")
                                for t3 in range(PAIRS):
                                    nc.tensor.matmul(
                                        pp[:, 0:ow],
                                        lhsT=at_chunk[:, t3, sub * P:(sub + 1) * P],
                                        rhs=wp_sb[t3][:, o0:o0 + ow],
                                        start=(t3 == 0), stop=(t3 == PAIRS - 1))
                                ost = outst_pool.tile([P, QC], F32, tag="ost",
                                                      name="ost")
                                nc.vector.tensor_copy(out=ost[:, 0:ow],
                                                      in_=pp[:, 0:ow])
                                n0 = qc * QC + sub * P
                                nc.sync.dma_start(out_d[n0:n0 + P, o0:o0 + ow],
                                                  ost[:, 0:ow])
                            pieces.append(piece)
                    return pieces

                # ---------------- attention loop ----------------

                AVLAG = 5  # AV trails exp by 5 k-tiles; the tail of each
                # loop's AV work carries into the next loop so the PE never
                # drains at loop boundaries (a >3.4us gap re-triggers HAM
                # throttling)

                def run_loop(pr, qc, fillers, sched):
                    qsl = slice(qc * QC, (qc + 1) * QC)
                    if pr == 0:
                        at_chunks[qc] = attn_pool.tile(
                            [P, PAIRS, QC], BF16, tag=f"attnT{qc}",
                            name="at_chunk")
                    at_chunk = at_chunks[qc]
                    av = ps_av.tile([P, 2, QC], F32, tag="av")
                    a_ts = {}
                    st = {}

                    def st_exp(kt):
                        sp = ps_s.tile([P, 2, QC], F32, tag="s")
                        for h2 in range(2):
                            nc.tensor.matmul(
                                sp[:, h2, :],
                                lhsT=k_sb[pr][h2 * DH:(h2 + 1) * DH,
                                              kt * P:(kt + 1) * P],
                                rhs=q_sb[pr][h2 * DH:(h2 + 1) * DH, qsl],
                                start=True, stop=True,
                                tile_position=(h2 * DH, 0))
                        a_t = aT_pool.tile([P, 2, QC], BF16, tag="aT")
                        with nc.allow_low_precision(reason="bf16 attn weights"):
                            nc.scalar.activation(
                                out=a_t[:, :, :],
                                in_=sp[:, :, :],
                                func=mybir.ActivationFunctionType.Exp,
                                bias=expb_sb[:, 0:1],
                                scale=float(SCALE))
                        a_ts[kt] = a_t

                    def av_mm(kt):
                        a_t = a_ts.pop(kt)
                        base = (kt * PAIRS + pr) * VB
                        for h2 in range(2):
                            # h0: M=128 window [Vh0|1|Vh1...]; h1: M=66 window
                            # [Vh1|pad|1] — both stay inside this (kt, pr)
                            # block; denominators land at rows 64 / 65
                            m = P if h2 == 0 else 2 * DH + 3 - (DH + 1)
                            nc.tensor.matmul(
                                av[0:m, h2, :],
                                lhsT=v_sb[:, base + h2 * (DH + 1):
                                          base + h2 * (DH + 1) + m],
                                rhs=a_t[:, h2, :],
                                start=(kt == 0), stop=(kt == KT - 1))

                    for kt in range(KT):
                        for f in fillers[kt]:
                            f()
                        for (pkt, pf) in sched:
                            if pkt == kt:
                                pf()
                        if kt >= AVLAG:
                            av_mm(kt - AVLAG)
                        st_exp(kt)

                    def evac():
                        # evacuate AV numerators + denominator rows (DVE,
                        # bf16); den_h0 at row 64 bank 0, den_h1 row 65 bank 1
                        avc = norm_pool.tile([DH + 2, 2, QC], BF16,
                                             tag=f"avc{pr}", name="avc")
                        st["avc"] = avc
                        with nc.allow_low_precision(reason="bf16 av"):
                            nc.vector.tensor_copy(out=avc[:, :, :],
                                                  in_=av[0:DH + 2, :, :])
                        # pack both denominators into one tile at base
                        # partitions 0 / 32 (the only legal matmul base
                        # partitions) so one reciprocal serves both heads
                        den_t = norm_pool.tile([33, QC], BF16,
                                               tag=f"dent{pr}", name="dent")
                        st["den"] = den_t
                        nc.gpsimd.dma_start(den_t[0:1, :], avc[DH:DH + 1, 0, :])
                        nc.gpsimd.dma_start(den_t[32:33, :],
                                          avc[DH + 1:DH + 2, 1, :])
                        # rows 1-31 are never consumed; fill them so the
                        # [33,:] reciprocal reads fully-initialized memory
                        nc.gpsimd.dma_start(den_t[1:32, :],
                                          avc[DH - 31:DH, 0, :])
                        if debug:
                            nc.sync.dma_start(den_dbg[pr, qc, :, :],
                                              den_t[:, :])

                    NREC = 8  # fine-grained so the in-order DVE queue never
                    # blocks the small ops that release aux psum slots

                    def recip(piece):
                        if piece == 0:
                            st["rc"] = norm_pool.tile([33, QC], BF16,
                                                      tag=f"rc{pr}", name="rc")
                        w = QC // NREC
                        o = piece * w
                        with nc.allow_low_precision(reason="softmax recip"):
                            nc.vector.reciprocal(
                                st["rc"][:, o:o + w],
                                st["den"][:, o:o + w])

                    def norm_h(h2):
                        avc, rc = st["avc"], st["rc"]
                        bc = aux.tile([P, QC], F32, tag="bc", name="bc")
                        nc.tensor.matmul(
                            bc[:, :],
                            lhsT=ones_sb[32 * h2:32 * h2 + 1, :],
                            rhs=rc[32 * h2:32 * h2 + 1, :],
                            start=True, stop=True)
                        if h2 == 0:
                            with nc.allow_low_precision(reason="bf16 attn out"):
                                nc.vector.tensor_mul(
                                    out=at_chunk[0:DH, pr, :],
                                    in0=avc[0:DH, 0, :],
                                    in1=bc[0:DH, :])
                        else:
                            tmp = norm_pool.tile([DH, QC], BF16,
                                                 tag=f"tmp{pr}", name="tmp")
                            with nc.allow_low_precision(reason="bf16 attn out"):
                                nc.vector.tensor_mul(
                                    out=tmp[:, :],
                                    in0=avc[0:DH, 1, :],
                                    in1=bc[0:DH, :])
                            nc.sync.dma_start(at_chunk[DH:P, pr, :], tmp[:, :])

                    items = []
                    for j in range(AVLAG - 1):
                        items.append((j, lambda kt=KT - AVLAG + j: av_mm(kt)))

                    def last_av_and_evac():
                        av_mm(KT - 1)
                        evac()

                    items.append((AVLAG - 1, last_av_and_evac))
                    items += [(6 + j // 2, lambda j=j: recip(j))
                              for j in range(8)]
                    # norm bc matmuls a FULL loop later (units 0-1 of
                    # loop+2): the PE queue runs ~4-6us ahead of wall time
                    items2 = [(0, lambda: norm_h(0)), (1, lambda: norm_h(1))]
                    return items, items2

                # ---------------- emission schedule ----------------

                def emit_all(pieces):
                    for p in pieces:
                        p()

                # prologue: just K(pr0, qc0) + Q(pr0, qc0) — the minimum for
                # L0's first S^T; the other K(pr0) chunks become L0 fillers
                emit_all(qk_pieces(1, 0, 0))
                emit_all(qk_pieces(0, 0, 0))

                # static filler plan: plan[(pr, qc)][kt] = list of closures
                plan = {(pr, qc): [[] for _ in range(KT)]
                        for pr in range(PAIRS) for qc in range(NQC)}

                def place(loop, slot0, pieces, stride=1):
                    s = slot0
                    for p in pieces:
                        plan[loop][s].append(p)
                        s += stride

                # L0 (0,0): K(pr0, qc) just-in-time for S^T(4qc) (the
                # xT chunk DMAs are still in flight early in L0); V at units
                # kt+2, kt+3 (consumed by AV at unit kt+AVLAG); the v(13..15)
                # tails spill into L1 units 0..2 just ahead of the carried AV
                for i in range(1, NQC):
                    place((0, 0), 4 * i - 2, qk_pieces(1, 0, i))
                for kt in range(KT):
                    ps = v_pieces(kt)
                    if kt <= 12:
                        place((0, 0), kt + 2, ps)
                    elif kt == 13:
                        place((0, 0), 15, [ps[0]])
                        place((0, 1), 0, [ps[1]])
                    else:
                        place((0, 1), kt - 14, ps)
                place((0, 0), 8, qk_pieces(0, 0, 1))
                # L1 (0,1): Q(0,2); K(pr1) x4; Q(pr1,0)
                place((0, 1), 2, qk_pieces(0, 0, 2))
                for i in range(NQC):
                    place((0, 1), 4 + 2 * i, qk_pieces(1, 1, i))
                place((0, 1), 12, qk_pieces(0, 1, 0))
                # L2 (0,2): Q(0,3); Q(pr1, 1..3)
                place((0, 2), 0, qk_pieces(0, 0, 3))
                for i in range(1, NQC):
                    place((0, 2), 2 * i, qk_pieces(0, 1, i))
                # L3 (0,3): K(pr2) x4; Q(pr2,0)
                for i in range(NQC):
                    place((0, 3), 2 * i, qk_pieces(1, 2, i))
                place((0, 3), 10, qk_pieces(0, 2, 0))
                # (1, 0..2): Q(pr2, 1..3)
                for i in range(1, NQC):
                    place((1, i - 1), 0, qk_pieces(0, 2, i))

                # pair loops alternate (1,qc),(2,qc) after pr0 so that
                # at_chunk(qc) — completed by the carried norm(2,qc) during
                # loop (1,qc+1) — feeds proj(qc) fillers in loop (2,qc+1),
                # a full loop after its normalization
                grid = [(0, qc) for qc in range(NQC)]
                for qc in range(NQC):
                    grid += [(1, qc), (2, qc)]
                # loop M consumes items1 of loop M-1 and items2 of loop M-2
                last1, last2, older2 = [], [], []
                for (pr, qc) in grid:
                    fillers = plan[(pr, qc)]
                    if pr == 2 and qc >= 1:
                        for j, p in enumerate(proj_pieces(qc - 1)):
                            fillers[8 + (7 * j) // 8].append(p)
                    i1, i2 = run_loop(pr, qc, fillers, last1 + older2)
                    older2 = last2
                    last1, last2 = i1, i2

                # epilogue: norm of the second-to-last loop, the last loop's
                # AV tail + evac + recip, its norm, then proj(qc3)
                for items in (older2, last1, last2):
                    for (_, pf) in sorted(items, key=lambda x: x[0]):
                        pf()
                emit_all(proj_pieces(NQC - 1))
                if debug:
                    for p in range(PAIRS):
                        nc.sync.dma_start(qk_dbg[0, p, :, :], q_sb[p][:, :])
                        nc.sync.dma_start(qk_dbg[1, p, :, :], k_sb[p][:, :])
                    nc.sync.dma_start(v_dbg[:, :], v_sb[:, :])
                    for qc in range(NQC):
                        nc.sync.dma_start(at_dbg[qc, :, :, :],
                                          at_chunks[qc][:, :, :])

    nc.finalize()
    return nc


_NC = None


def _get_nc():
    global _NC
    if _NC is None:
        _NC = build_nc()
    return _NC


def _bf16(a):
    return np.ascontiguousarray(a).astype(ml_dtypes.bfloat16)


def _make_in_maps(inputs):
    x = np.asarray(inputs["x"], dtype=np.float32)
    w_qkv = np.asarray(inputs["w_qkv"], dtype=np.float32)
    b_qkv = np.asarray(inputs["b_qkv"], dtype=np.float32)
    w_proj = np.asarray(inputs["w_proj"], dtype=np.float32)

    in_maps = []
    for c in range(8):
        b, g = c // 2, c % 2
        sl = slice(384 * g, 384 * g + 384)
        xT = np.ascontiguousarray(x[b].T)                       # [768, 2048]
        wq = w_qkv[0:768][sl]                                    # [384, 768]
        wk = w_qkv[768:1536][sl]
        wv = w_qkv[1536:2304][sl]
        wqkvT = np.ascontiguousarray(np.concatenate([wq, wk, wv], axis=0).T)
        bq = b_qkv[0:768][sl]
        bk = b_qkv[768:1536][sl]
        bv = b_qkv[1536:2304][sl]
        b_qk = np.ascontiguousarray(
            np.concatenate([bq, bk]).reshape(6, P).T)            # [128, 6]
        wpT = np.ascontiguousarray(w_proj[:, sl].T)
        in_maps.append({
            "ones": np.ones((P, P), dtype=ml_dtypes.bfloat16),
            "xT": _bf16(xT),
            "wqkvT": _bf16(wqkvT),
            "b_qk": b_qk,
            "b_v": _bf16(bv.reshape(1, 384)),
            "wpT": _bf16(wpT),
        })
    return in_maps


def _run(inputs, trace=False):
    nc = _get_nc()
    in_maps = _make_in_maps(inputs)
    res = run_bass_kernel_spmd(nc, in_maps, core_ids=list(range(8)), trace=trace)
    b_proj = np.asarray(inputs["b_proj"], dtype=np.float32)
    out = np.empty((4, NQ, CD), dtype=np.float32)
    for b in range(4):
        out[b] = res.results[2 * b]["out"] + res.results[2 * b + 1]["out"] + b_proj
    return out, res


def kernel(**inputs) -> np.ndarray:
    out, _ = _run(inputs, trace=False)
    return out


# revision 40
# speedup vs baseline: 1.0429x; 1.0429x over previous
"""Multi-head attention (B=4, N=2048, D=768, H=12) on 8 trn2 NeuronCores.

Sharding: core c -> (batch b = c//2, head-half g = c%2).  Each core computes
the qkv projection for its 6 heads, attention, and a partial output
projection (over its 384 feature columns).  The host sums the two partials
per batch and adds the proj bias.  No collectives.

v3 design notes (from HW microbenchmarks):
 - Matmuls stream 1 cyc/col at 2.4 GHz only while the PE is continuously
   fed; idle gaps drop the PE to a lower p-state (~2x) and >3.4us gaps
   re-trigger HAM throttling.  The attention inner loop is ACT-bound
   (exp [128,1024] = 1114 ns vs ~639 ns of PE work per k-tile), so all
   non-attention matmuls (QKV, V, proj) are interleaved as fine-grained
   fillers inside the attention loops to keep the PE warm and busy.
 - Loop grid is pair-major: for pr in 0..2: for qc in 0..3.  Fillers are
   placed with explicit deadlines (Q/K of pair p+1 finish during pair p's
   loops; proj(qc) runs during (pr2, qc+1)).
 - All matmul operands bf16 (PSUM stays fp32): halves SBUF traffic + DMA.
 - V stored flat per (kt, pair) as [Vh0(64)|1|Vh1(64)|pad|1] (131 wide);
   AV uses M=128 lhsT windows (h0: +0, h1: +65) so FWL stays on, softmax
   denominators appear at PSUM row 64 (h0) / row 65 (h1), and junk rows
   are never read.  Denominators are repacked to partitions 64/65 of one
   bank so ONE [2,512] DVE reciprocal serves both heads.
 - proj results DMA directly PSUM -> DRAM (no SBUF staging).
 - PSUM: 2x S^T double-buffer (4 banks) + AV accum (2) + aux pool (2).
"""

import numpy as np
import ml_dtypes

import concourse.bacc as bacc
import concourse.bass as bass  # noqa: F401
import concourse.mybir as mybir
import concourse.tile as tile
from concourse.bass_utils import run_bass_kernel_spmd

P = 128
NQ = 2048          # sequence length
CD = 768           # model dim
NHC = 6            # heads per core
DH = 64            # head dim
SCALE = DH ** -0.5
CT = CD // P       # 6 c-tiles
KT = NQ // P       # 16 k-tiles
QC = 512           # q chunk
NQC = NQ // QC     # 4
PAIRS = NHC // 2   # 3
VB = 2 * DH + 3    # 131: flat v block width per (kt, pair)
VW = KT * PAIRS * VB + DH + 2  # + tail pad so the last h1 window is in-bounds

F32 = mybir.dt.float32
BF16 = mybir.dt.bfloat16


def build_nc(n_reps=1, debug=False):
    nc = bacc.Bacc("TRN2", debug=False, num_devices=8)

    xT_d = nc.dram_tensor("xT", [CD, NQ], BF16, kind="ExternalInput")
    wqkvT_d = nc.dram_tensor("wqkvT", [CD, 3 * 384], BF16, kind="ExternalInput")
    bqk_d = nc.dram_tensor("b_qk", [P, 6], F32, kind="ExternalInput")
    bv_d = nc.dram_tensor("b_v", [1, 384], BF16, kind="ExternalInput")
    wpT_d = nc.dram_tensor("wpT", [384, CD], BF16, kind="ExternalInput")
    ones_d = nc.dram_tensor("ones", [P, P], BF16, kind="ExternalInput")
    out_d = nc.dram_tensor("out", [NQ, CD], F32, kind="ExternalOutput")
    if debug:
        qk_dbg = nc.dram_tensor("qk_dbg", [2, PAIRS, P, NQ], BF16,
                                kind="ExternalOutput")
        v_dbg = nc.dram_tensor("v_dbg", [P, VW], BF16, kind="ExternalOutput")
        at_dbg = nc.dram_tensor("at_dbg", [NQC, P, PAIRS, QC], BF16,
                                kind="ExternalOutput")
        den_dbg = nc.dram_tensor("den_dbg", [PAIRS, NQC, 33, QC], BF16,
                                 kind="ExternalOutput")

    with tile.TileContext(nc) as tc:
        with (
            tc.tile_pool(name="consts", bufs=1) as consts,
            tc.tile_pool(name="big", bufs=1) as big,
            tc.tile_pool(name="attn", bufs=1) as attn_pool,
            tc.tile_pool(name="aT", bufs=4) as aT_pool,
            tc.tile_pool(name="norm", bufs=1) as norm_pool,
            tc.tile_pool(name="outst", bufs=2) as outst_pool,
            tc.tile_pool(name="ps_s", bufs=2, space="PSUM") as ps_s,
            tc.tile_pool(name="ps_av", bufs=1, space="PSUM") as ps_av,
            tc.tile_pool(name="aux", bufs=2, space="PSUM") as aux,
        ):
            # ---- constants ----
            # single tiles with a c-tile dim; DMA issue costs ~650ns each on
            # ANY queue, so batch the input loads into a few big DMAs (one
            # per xT chunk / weight piece) using partition-regrouped dram
            # views, alternating the sync/gpsimd rings
            xT_all = consts.tile([P, CT, NQ], BF16, tag="xT", name="xT_all")
            wq_all = consts.tile([P, CT, 3 * 384], BF16, tag="wq", name="wq_all")
            xT_sb = [xT_all[:, ct, :] for ct in range(CT)]
            wq_sb = [wq_all[:, ct, :] for ct in range(CT)]
            xTv = xT_d.rearrange("(a p) n -> p a n", p=P)
            wqv = wqkvT_d.rearrange("(a p) n -> p a n", p=P)
            nc.gpsimd.dma_start(wq_all[:, :, 384:768], wqv[:, :, 384:768])
            nc.sync.dma_start(xT_all[:, :, 0:QC], xTv[:, :, 0:QC])
            nc.gpsimd.dma_start(wq_all[:, :, 0:384], wqv[:, :, 0:384])
            nc.sync.dma_start(wq_all[:, :, 768:1152], wqv[:, :, 768:1152])
            nc.gpsimd.dma_start(xT_all[:, :, QC:2 * QC], xTv[:, :, QC:2 * QC])
            nc.sync.dma_start(xT_all[:, :, 2 * QC:3 * QC],
                              xTv[:, :, 2 * QC:3 * QC])
            nc.gpsimd.dma_start(xT_all[:, :, 3 * QC:4 * QC],
                                xTv[:, :, 3 * QC:4 * QC])
            wp_all = consts.tile([P, 3, CD], BF16, tag="wpT", name="wp_all")
            wp_sb = [wp_all[:, t3, :] for t3 in range(3)]
            nc.sync.dma_start(
                wp_all[:, :, :], wpT_d.rearrange("(a p) n -> p a n", p=P))
            bqk_sb = consts.tile([P, 6], F32, tag="bqk")
            nc.gpsimd.dma_start(bqk_sb[:, :], bqk_d[:, :])
            bv_sb = consts.tile([1, 384], BF16, tag="bv")
            nc.gpsimd.dma_start(bv_sb[:, :], bv_d[:, :])
            ones_sb = consts.tile([P, P], BF16, tag="ones")
            nc.gpsimd.dma_start(ones_sb[:, :], ones_d[:, :])
            # -ln(64) exp bias (softmax-invariant; keeps 1/sum well scaled)
            expb_sb = consts.tile([P, 1], F32, tag="expb")
            nc.vector.memset(expb_sb[:, :], -4.1588830833596715)

            for _rep in range(n_reps):
                # ---- persistent activations ----
                # per-pair Q^T/K^T [128, 2048]: rows 0-63 head 2p, 64-127 head 2p+1
                q_sb = [big.tile([P, NQ], BF16, tag=f"q{p}", name=f"q{p}")
                        for p in range(PAIRS)]
                k_sb = [big.tile([P, NQ], BF16, tag=f"k{p}", name=f"k{p}")
                        for p in range(PAIRS)]
                # flat v: per (kt, pair) block [Vh0|1|Vh1|pad|1], width 131
                v_sb = big.tile([P, VW], BF16, tag="v")
                nc.vector.memset(v_sb[:, :], 0.0)
                vv = v_sb[:, 0:KT * PAIRS * VB].rearrange(
                    "p (a b w) -> p a b w", a=KT, b=PAIRS)
                # ones columns for the softmax denominators
                nc.gpsimd.dma_start(
                    vv[:, :, :, DH],
                    ones_d[:, 0:KT * PAIRS].rearrange("p (a b) -> p a b", a=KT))
                nc.gpsimd.dma_start(
                    vv[:, :, :, 2 * DH + 2],
                    ones_d[:, 0:KT * PAIRS].rearrange("p (a b) -> p a b", a=KT))

                # ---------------- building blocks ----------------

                def qk_pieces(kind, t, qc):
                    # Q^T (kind 0) / K^T (kind 1) pair-tile t, one 512-chunk,
                    # split into two 3-matmul pieces sharing one aux psum.
                    dest = (q_sb if kind == 0 else k_sb)[t]
                    col0 = kind * 384 + t * P
                    st = {}

                    def p1():
                        st["ps"] = aux.tile([P, QC], F32, tag="aux", name="qkvps")
                        for ct in range(3):
                            nc.tensor.matmul(
                                st["ps"][:, :],
                                lhsT=wq_sb[ct][:, col0:col0 + P],
                                rhs=xT_sb[ct][:, qc * QC:(qc + 1) * QC],
                                start=(ct == 0), stop=False)

                    def p2():
                        for ct in range(3, CT):
                            nc.tensor.matmul(
                                st["ps"][:, :],
                                lhsT=wq_sb[ct][:, col0:col0 + P],
                                rhs=xT_sb[ct][:, qc * QC:(qc + 1) * QC],
                                start=False, stop=(ct == CT - 1))
                        with nc.allow_low_precision(reason="bf16 q/k"):
                            nc.vector.tensor_scalar_add(
                                out=dest[:, qc * QC:(qc + 1) * QC],
                                in0=st["ps"][:, :],
                                scalar1=bqk_sb[:, kind * 3 + t:kind * 3 + t + 1])

                    return [p1, p2]

                def v_pieces(nt):
                    st = {}

                    def p1():
                        st["ps"] = aux.tile([P, QC], F32, tag="aux", name="qkvps")
                        for ct in range(3):
                            nc.tensor.matmul(
                                st["ps"][:, 0:384],
                                lhsT=xT_sb[ct][:, nt * P:(nt + 1) * P],
                                rhs=wq_sb[ct][:, 768:1152],
                                start=(ct == 0), stop=False)

                    def p2():
                        for ct in range(3, CT):
                            nc.tensor.matmul(
                                st["ps"][:, 0:384],
                                lhsT=xT_sb[ct][:, nt * P:(nt + 1) * P],
                                rhs=wq_sb[ct][:, 768:1152],
                                start=False, stop=False)
                        nc.tensor.matmul(
                            st["ps"][:, 0:384],
                            lhsT=ones_sb[0:1, :],
                            rhs=bv_sb[0:1, :],
                            start=False, stop=True)
                        src = st["ps"][:, 0:384].rearrange(
                            "p (b c d) -> p b c d", b=PAIRS, c=2)
                        with nc.allow_low_precision(reason="bf16 v"):
                            nc.vector.tensor_copy(
                                out=vv[:, nt, :, 0:DH],
                                in_=src[:, :, 0, :])
                            nc.vector.tensor_copy(
                                out=vv[:, nt, :, DH + 1:2 * DH + 1],
                                in_=src[:, :, 1, :])

                    return [p1, p2]

                at_chunks = {}

                def proj_pieces(qc):
                    at_chunk = at_chunks[qc]
                    pieces = []
                    for sub in range(QC // P):
                        for (o0, ow) in ((0, 512), (512, 256)):
                            def piece(sub=sub, o0=o0, ow=ow):
                                pp = aux.tile([P, QC], F32, tag="aux")
                                for t3 in range(PAIRS):
                                    nc.tensor.matmul(
                                        pp[:, 0:ow],
                                        lhsT=at_chunk[:, t3, sub * P:(sub + 1) * P],
                                        rhs=wp_sb[t3][:, o0:o0 + ow],
                                        start=(t3 == 0), stop=(t3 == PAIRS - 1))
                                ost = outst_pool.tile([P, QC], F32, tag="ost",
                                                      name="ost")
                                nc.vector.tensor_copy(out=ost[:, 0:ow],
                                                      in_=pp[:, 0:ow])
                                n0 = qc * QC + sub * P
                                nc.sync.dma_start(out_d[n0:n0 + P, o0:o0 + ow],
                                                  ost[:, 0:ow])
                            pieces.append(piece)
                    return pieces

                # ---------------- attention loop ----------------

                AVLAG = 5  # AV trails exp by 5 k-tiles; the tail of each
                # loop's AV work carries into the next loop so the PE never
                # drains at loop boundaries (a >3.4us gap re-triggers HAM
                # throttling)

                def run_loop(pr, qc, fillers, sched):
                    qsl = slice(qc * QC, (qc + 1) * QC)
                    if pr == 0:
                        at_chunks[qc] = attn_pool.tile(
                            [P, PAIRS, QC], BF16, tag=f"attnT{qc}",
                            name="at_chunk")
                    at_chunk = at_chunks[qc]
                    av = ps_av.tile([P, 2, QC], F32, tag="av")
                    a_ts = {}
                    st = {}

                    def st_exp(kt):
                        sp = ps_s.tile([P, 2, QC], F32, tag="s")
                        for h2 in range(2):
                            nc.tensor.matmul(
                                sp[:, h2, :],
                                lhsT=k_sb[pr][h2 * DH:(h2 + 1) * DH,
                                              kt * P:(kt + 1) * P],
                                rhs=q_sb[pr][h2 * DH:(h2 + 1) * DH, qsl],
                                start=True, stop=True,
                                tile_position=(h2 * DH, 0))
                        a_t = aT_pool.tile([P, 2, QC], BF16, tag="aT")
                        with nc.allow_low_precision(reason="bf16 attn weights"):
                            nc.scalar.activation(
                                out=a_t[:, :, :],
                                in_=sp[:, :, :],
                                func=mybir.ActivationFunctionType.Exp,
                                bias=expb_sb[:, 0:1],
                                scale=float(SCALE))
                        a_ts[kt] = a_t

                    def av_mm(kt):
                        a_t = a_ts.pop(kt)
                        base = (kt * PAIRS + pr) * VB
                        for h2 in range(2):
                            # h0: M=128 window [Vh0|1|Vh1...]; h1: M=66 window
                            # [Vh1|pad|1] — both stay inside this (kt, pr)
                            # block; denominators land at rows 64 / 65
                            m = P if h2 == 0 else 2 * DH + 3 - (DH + 1)
                            nc.tensor.matmul(
                                av[0:m, h2, :],
                                lhsT=v_sb[:, base + h2 * (DH + 1):
                                          base + h2 * (DH + 1) + m],
                                rhs=a_t[:, h2, :],
                                start=(kt == 0), stop=(kt == KT - 1))

                    for kt in range(KT):
                        for f in fillers[kt]:
                            f()
                        for (pkt, pf) in sched:
                            if pkt == kt:
                                pf()
                        if kt >= AVLAG:
                            av_mm(kt - AVLAG)
                        st_exp(kt)

                    def evac():
                        # evacuate AV numerators + denominator rows (DVE,
                        # bf16); den_h0 at row 64 bank 0, den_h1 row 65 bank 1
                        avc = norm_pool.tile([DH + 2, 2, QC], BF16,
                                             tag=f"avc{pr}", name="avc")
                        st["avc"] = avc
                        with nc.allow_low_precision(reason="bf16 av"):
                            nc.vector.tensor_copy(out=avc[:, :, :],
                                                  in_=av[0:DH + 2, :, :])
                        # pack both denominators into one tile at base
                        # partitions 0 / 32 (the only legal matmul base
                        # partitions) so one reciprocal serves both heads
                        den_t = norm_pool.tile([33, QC], BF16,
                                               tag=f"dent{pr}", name="dent")
                        st["den"] = den_t
                        nc.gpsimd.dma_start(den_t[0:1, :], avc[DH:DH + 1, 0, :])
                        nc.gpsimd.dma_start(den_t[32:33, :],
                                          avc[DH + 1:DH + 2, 1, :])
                        # rows 1-31 are never consumed; fill them so the
                        # [33,:] reciprocal reads fully-initialized memory
                        nc.gpsimd.dma_start(den_t[1:32, :],
                                          avc[DH - 31:DH, 0, :])
                        if debug:
                            nc.sync.dma_start(den_dbg[pr, qc, :, :],
                                              den_t[:, :])

                    NREC = 8  # fine-grained so the in-order DVE queue never
                    # blocks the small ops that release aux psum slots

                    def recip(piece):
                        if piece == 0:
                            st["rc"] = norm_pool.tile([33, QC], BF16,
                                                      tag=f"rc{pr}", name="rc")
                        w = QC // NREC
                        o = piece * w
                        with nc.allow_low_precision(reason="softmax recip"):
                            nc.vector.reciprocal(
                                st["rc"][:, o:o + w],
                                st["den"][:, o:o + w])

                    def norm_h(h2):
                        avc, rc = st["avc"], st["rc"]
                        bc = aux.tile([P, QC], F32, tag="aux", name="bc")
                        nc.tensor.matmul(
                            bc[:, :],
                            lhsT=ones_sb[32 * h2:32 * h2 + 1, :],
                            rhs=rc[32 * h2:32 * h2 + 1, :],
                            start=True, stop=True)
                        if h2 == 0:
                            with nc.allow_low_precision(reason="bf16 attn out"):
                                nc.vector.tensor_mul(
                                    out=at_chunk[0:DH, pr, :],
                                    in0=avc[0:DH, 0, :],
                                    in1=bc[0:DH, :])
                        else:
                            tmp = norm_pool.tile([DH, QC], BF16,
                                                 tag=f"tmp{pr}", name="tmp")
                            with nc.allow_low_precision(reason="bf16 attn out"):
                                nc.vector.tensor_mul(
                                    out=tmp[:, :],
                                    in0=avc[0:DH, 1, :],
                                    in1=bc[0:DH, :])
                            nc.sync.dma_start(at_chunk[DH:P, pr, :], tmp[:, :])

                    items = []
                    for j in range(AVLAG - 1):
                        items.append((j, lambda kt=KT - AVLAG + j: av_mm(kt)))

                    def last_av_and_evac():
                        av_mm(KT - 1)
                        evac()

                    items.append((AVLAG - 1, last_av_and_evac))
                    items += [(6 + j // 2, lambda j=j: recip(j))
                              for j in range(8)]
                    # norm bc matmuls a FULL loop later (units 0-1 of
                    # loop+2): the PE queue runs ~4-6us ahead of wall time
                    items2 = [(0, lambda: norm_h(0)), (1, lambda: norm_h(1))]
                    return items, items2

                # ---------------- emission schedule ----------------

                def emit_all(pieces):
                    for p in pieces:
                        p()

                # prologue: just K(pr0, qc0) + Q(pr0, qc0) — the minimum for
                # L0's first S^T; the other K(pr0) chunks become L0 fillers
                emit_all(qk_pieces(1, 0, 0))
                emit_all(qk_pieces(0, 0, 0))

                # static filler plan: plan[(pr, qc)][kt] = list of closures
                plan = {(pr, qc): [[] for _ in range(KT)]
                        for pr in range(PAIRS) for qc in range(NQC)}

                def place(loop, slot0, pieces, stride=1):
                    s = slot0
                    for p in pieces:
                        plan[loop][s].append(p)
                        s += stride

                # L0 (0,0): K(pr0, qc) just-in-time for S^T(4qc) (the
                # xT chunk DMAs are still in flight early in L0); V at units
                # kt+2, kt+3 (consumed by AV at unit kt+AVLAG); the v(13..15)
                # tails spill into L1 units 0..2 just ahead of the carried AV
                for i in range(1, NQC):
                    place((0, 0), 4 * i - 2, qk_pieces(1, 0, i))
                for kt in range(KT):
                    ps = v_pieces(kt)
                    if kt <= 12:
                        place((0, 0), kt + 2, ps)
                    elif kt == 13:
                        place((0, 0), 15, [ps[0]])
                        place((0, 1), 0, [ps[1]])
                    else:
                        place((0, 1), kt - 14, ps)
                place((0, 0), 8, qk_pieces(0, 0, 1))
                # L1 (0,1): Q(0,2); K(pr1) x4; Q(pr1,0)
                place((0, 1), 2, qk_pieces(0, 0, 2))
                for i in range(NQC):
                    place((0, 1), 4 + 2 * i, qk_pieces(1, 1, i))
                place((0, 1), 12, qk_pieces(0, 1, 0))
                # L2 (0,2): Q(0,3); Q(pr1, 1..3)
                place((0, 2), 0, qk_pieces(0, 0, 3))
                for i in range(1, NQC):
                    place((0, 2), 2 * i, qk_pieces(0, 1, i))
                # L3 (0,3): K(pr2) x4; Q(pr2,0)
                for i in range(NQC):
                    place((0, 3), 2 * i, qk_pieces(1, 2, i))
                place((0, 3), 10, qk_pieces(0, 2, 0))
                # (1, 0..2): Q(pr2, 1..3)
                for i in range(1, NQC):
                    place((1, i - 1), 0, qk_pieces(0, 2, i))

                # pair loops alternate (1,qc),(2,qc) after pr0 so that
                # at_chunk(qc) — completed by the carried norm(2,qc) during
                # loop (1,qc+1) — feeds proj(qc) fillers in loop (2,qc+1),
                # a full loop after its normalization
                grid = [(0, qc) for qc in range(NQC)]
                for qc in range(NQC):
                    grid += [(1, qc), (2, qc)]
                # loop M consumes items1 of loop M-1 and items2 of loop M-2
                last1, last2, older2 = [], [], []
                for (pr, qc) in grid:
                    fillers = plan[(pr, qc)]
                    if pr == 2 and qc >= 1:
                        for j, p in enumerate(proj_pieces(qc - 1)):
                            fillers[2 + (3 * j) // 4].append(p)
                    i1, i2 = run_loop(pr, qc, fillers, last1 + older2)
                    older2 = last2
                    last1, last2 = i1, i2

                # epilogue: norm of the second-to-last loop, the last loop's
                # AV tail + evac + recip, its norm, then proj(qc3)
                for items in (older2, last1, last2):
                    for (_, pf) in sorted(items, key=lambda x: x[0]):
                        pf()
                emit_all(proj_pieces(NQC - 1))
                if debug:
                    for p in range(PAIRS):
                        nc.sync.dma_start(qk_dbg[0, p, :, :], q_sb[p][:, :])
                        nc.sync.dma_start(qk_dbg[1, p, :, :], k_sb[p][:, :])
                    nc.sync.dma_start(v_dbg[:, :], v_sb[:, :])
                    for qc in range(NQC):
                        nc.sync.dma_start(at_dbg[qc, :, :, :],
                                          at_chunks[qc][:, :, :])

    nc.finalize()
    return nc


_NC = None


def _get_nc():
    global _NC
    if _NC is None:
        _NC = build_nc()
    return _NC


def _bf16(a):
    return np.ascontiguousarray(a).astype(ml_dtypes.bfloat16)


def _make_in_maps(inputs):
    x = np.asarray(inputs["x"], dtype=np.float32)
    w_qkv = np.asarray(inputs["w_qkv"], dtype=np.float32)
    b_qkv = np.asarray(inputs["b_qkv"], dtype=np.float32)
    w_proj = np.asarray(inputs["w_proj"], dtype=np.float32)

    in_maps = []
    for c in range(8):
        b, g = c // 2, c % 2
        sl = slice(384 * g, 384 * g + 384)
        xT = np.ascontiguousarray(x[b].T)                       # [768, 2048]
        wq = w_qkv[0:768][sl]                                    # [384, 768]
        wk = w_qkv[768:1536][sl]
        wv = w_qkv[1536:2304][sl]
        wqkvT = np.ascontiguousarray(np.concatenate([wq, wk, wv], axis=0).T)
        bq = b_qkv[0:768][sl]
        bk = b_qkv[768:1536][sl]
        bv = b_qkv[1536:2304][sl]
        b_qk = np.ascontiguousarray(
            np.concatenate([bq, bk]).reshape(6, P).T)            # [128, 6]
        wpT = np.ascontiguousarray(w_proj[:, sl].T)
        in_maps.append({
            "ones": np.ones((P, P), dtype=ml_dtypes.bfloat16),
            "xT": _bf16(xT),
            "wqkvT": _bf16(wqkvT),
            "b_qk": b_qk,
            "b_v": _bf16(bv.reshape(1, 384)),
            "wpT": _bf16(wpT),
        })
    return in_maps


def _run(inputs, trace=False):
    nc = _get_nc()
    in_maps = _make_in_maps(inputs)
    res = run_bass_kernel_spmd(nc, in_maps, core_ids=list(range(8)), trace=trace)
    b_proj = np.asarray(inputs["b_proj"], dtype=np.float32)
    out = np.empty((4, NQ, CD), dtype=np.float32)
    for b in range(4):
        out[b] = res.results[2 * b]["out"] + res.results[2 * b + 1]["out"] + b_proj
    return out, res


def kernel(**inputs) -> np.ndarray:
    out, _ = _run(inputs, trace=False)
    return out
